# revision 1
# baseline (speedup 1.0000x reference)
"""MoE top-1 routing kernel for Trainium2, expert-parallel across 8 NeuronCores.

Strategy (per spec sharding hint): one expert per core. The (tiny) router
runs on host in fp64; tokens are dispatched host-side to their expert's
core (this is the all-to-all dispatch, done during input sharding). Each
core runs a dense FFN  y = gelu(x @ W1 + b1) @ W2  over its tokens in a
fully transposed dataflow:

    hT = W1^T @ xT        (lhsT = W1 slices, rhs = xT slices)
    yT = W2^T @ gelu(hT)  (lhsT = W2 slices, rhs = hT slices)

so the weight matrices are used directly as the stationary operand and no
on-device transposes are needed. Matmuls are bf16 with fp32 PSUM
accumulation; gelu (exact/erf) fused with the b1 bias on the scalar engine.
Outputs are combined host-side (the all-to-all combine) with b2 added on
host.

Shapes are hardcoded for the problem instance:
  x [4,2048,1024] f32, w1 [8,1024,4096], w2 [8,4096,1024], E=8 experts.
"""

import os
import sys

import numpy as np

sys.path.insert(0, "/opt/trn_rl_repo")

import ml_dtypes

try:
    from scipy.special import erf as _erf
except ImportError:          # pragma: no cover
    import math
    _erf = np.vectorize(math.erf)

import concourse.bass as bass
import concourse.mybir as mybir
import concourse.tile as tile
from concourse import bacc
from concourse import bass_utils

B, T, C = 4, 2048, 1024
H, E = 4096, 8
N_TOK = B * T
P = 128                      # partition dim
CAP = 1024                   # per-expert device token capacity; overflow tokens
# (counts above CAP; ~171 for this input) are computed exactly on host
# token phases, each split into 2 blocks that share one LDWEIGHTS per weight
PHASES = [(0, (256, 256)), (512, (256, 256))]
KC = C // P                  # 8  k-tiles over C
KH = H // P                  # 32 k-tiles over H
MH = H // P                  # 32 m-tiles over H (MM1 output partitions)
MC = C // P                  # 8  m-tiles over C (MM2 output partitions)

BF16 = mybir.dt.bfloat16
F32 = mybir.dt.float32

_COMPILED = None   # (nc, names) cache so repeat kernel() calls skip rebuild
LAST_RESULTS = None  # bass_utils.BassKernelResults of the last run (for test.py)


def _build_program():
    """Build the single-core Bass/Tile program (SPMD: same program, 8 cores)."""
    nc = bacc.Bacc(
        "TRN2",
        target_bir_lowering=False,
        debug=False,
        enable_asserts=False,
        num_devices=E,
    )

    xT_d = nc.dram_tensor("xt_in", [C, CAP], BF16, kind="ExternalInput").ap()
    w1_d = nc.dram_tensor("w1_in", [C, H], BF16, kind="ExternalInput").ap()
    w2_d = nc.dram_tensor("w2_in", [H, C], BF16, kind="ExternalInput").ap()
    b1_d = nc.dram_tensor("b1_in", [P, MH], F32, kind="ExternalInput").ap()
    yT_d = nc.dram_tensor("yt_out", [C, CAP], F32, kind="ExternalOutput").ap()

    with tile.TileContext(nc) as tc:
        with (
            tc.tile_pool(name="weights", bufs=1) as wpool,
            tc.tile_pool(name="xt", bufs=1) as xpool,
            tc.tile_pool(name="ht", bufs=1) as hpool,
            tc.tile_pool(name="yout", bufs=4) as ypool,
            tc.tile_pool(name="ps1", bufs=5, space=bass.MemorySpace.PSUM) as ps1pool,
            tc.tile_pool(name="ps2", bufs=3, space=bass.MemorySpace.PSUM) as ps2pool,
        ):
            # --- HAM warm-up: keep the PE busy through the ~25us weight-load
            # ramp so the clock-gate is at 2.4 GHz (and stays there) when the
            # first real matmul group becomes runnable ---
            warm = xpool.tile([P, 512], BF16, tag="warm")
            nc.vector.memset(warm[:], 0.0)
            wps = ps2pool.tile([P, 512], F32, tag="ps2")
            for _ in range(110):
                nc.tensor.matmul(wps[:], warm[:, :P], warm[:], start=True, stop=True)
            # prime the scalar engine's gelu LUT during the ramp so the
            # first real activation doesn't stall on ACT_TABLE_LOAD
            wact = hpool.tile([P, 8], BF16, tag="wact")
            nc.scalar.activation(wact[:], warm[:, :8],
                                 mybir.ActivationFunctionType.Gelu)

            # --- load everything on-chip once ---
            xT_sb = []
            for k in range(KC):
                t = xpool.tile([P, CAP], BF16, tag=f"xt{k}")
                nc.sync.dma_start(t[:], xT_d[k * P:(k + 1) * P, :])
                xT_sb.append(t)
            w1_sb = []
            for k in range(KC):
                t = wpool.tile([P, H], BF16, tag=f"w1_{k}")
                nc.sync.dma_start(t[:], w1_d[k * P:(k + 1) * P, :])
                w1_sb.append(t)
            b1_sb = wpool.tile([P, MH], F32, tag="b1")
            nc.sync.dma_start(b1_sb[:], b1_d[:])
            w2_sb = []
            for k in range(KH):
                t = wpool.tile([P, C], BF16, tag=f"w2_{k}")
                nc.sync.dma_start(t[:], w2_d[k * P:(k + 1) * P, :])
                w2_sb.append(t)

            # --- per phase: MM1+gelu -> hT, then MM2 -> yT. Within a phase,
            # the 2 token blocks are innermost so both matmuls reuse one
            # LDWEIGHTS per (m,k) weight tile (keeps the load hidden). ---
            for p0, blocks in PHASES:
                offs = []
                o = p0
                for tn in blocks:
                    offs.append((o, tn))
                    o += tn
                hT = {}
                for m in range(MH):
                    pss = [ps1pool.tile([P, tn], F32, tag="ps1", name=f"ps1_{m}_{i}")
                           for i, (_, tn) in enumerate(offs)]
                    for k in range(KC):
                        for bi, (t0, tn) in enumerate(offs):
                            nc.tensor.matmul(
                                pss[bi][:],
                                w1_sb[k][:, m * P:(m + 1) * P],
                                xT_sb[k][:, t0:t0 + tn],
                                start=(k == 0),
                                stop=(k == KC - 1),
                            )
                    for bi, (t0, tn) in enumerate(offs):
                        h = hpool.tile([P, tn], BF16, tag=f"h{m}_{bi}")
                        nc.scalar.activation(
                            h[:], pss[bi][:],
                            mybir.ActivationFunctionType.Gelu,
                            bias=b1_sb[:, m:m + 1],
                        )
                        hT[m, bi] = h
                for mc in range(MC):
                    pss = [ps2pool.tile([P, tn], F32, tag="ps2", name=f"ps2_{mc}_{i}")
                           for i, (_, tn) in enumerate(offs)]
                    for kh in range(KH):
                        for bi in range(len(offs)):
                            nc.tensor.matmul(
                                pss[bi][:],
                                w2_sb[kh][:, mc * P:(mc + 1) * P],
                                hT[kh, bi][:],
                                start=(kh == 0),
                                stop=(kh == KH - 1),
                            )
                    for bi, (t0, tn) in enumerate(offs):
                        y = ypool.tile([P, tn], F32, tag="y")
                        nc.vector.tensor_copy(y[:], pss[bi][:])
                        # gpsimd (SWDGE) queue: keeps output stores off the
                        # input load queue so w2 slices aren't delayed
                        nc.gpsimd.dma_start(
                            yT_d[mc * P:(mc + 1) * P, t0:t0 + tn], y[:])

    nc.compile()
    return nc


def kernel(x, w_router, b_router, w1, b1, w2, b2):
    global _COMPILED, LAST_RESULTS

    x = np.asarray(x, dtype=np.float32)
    w_router = np.asarray(w_router, dtype=np.float32)
    b_router = np.asarray(b_router, dtype=np.float32)
    w1 = np.asarray(w1, dtype=np.float32)
    b1 = np.asarray(b1, dtype=np.float32)
    w2 = np.asarray(w2, dtype=np.float32)
    b2 = np.asarray(b2, dtype=np.float32)

    # --- host router (fp64 for a faithful argmax) + top-1 dispatch ---
    X = x.reshape(N_TOK, C)
    logits = X.astype(np.float64) @ w_router.astype(np.float64) + b_router
    top1 = np.argmax(logits, axis=-1)
    idx_all = [np.nonzero(top1 == e)[0] for e in range(E)]
    idx = [i[:CAP] for i in idx_all]          # device share
    spill = [i[CAP:] for i in idx_all]        # host-computed overflow
    counts = [len(i) for i in idx]

    in_maps = []
    for e in range(E):
        xT = np.zeros((C, CAP), dtype=ml_dtypes.bfloat16)
        xT[:, :counts[e]] = X[idx[e]].T.astype(ml_dtypes.bfloat16)
        in_maps.append({
            "xt_in": xT,
            "w1_in": np.ascontiguousarray(w1[e]).astype(ml_dtypes.bfloat16),
            "w2_in": np.ascontiguousarray(w2[e]).astype(ml_dtypes.bfloat16),
            "b1_in": np.ascontiguousarray(b1[e].reshape(MH, P).T),
        })

    if _COMPILED is None:
        _COMPILED = _build_program()
    nc = _COMPILED

    LAST_RESULTS = bass_utils.run_bass_kernel_spmd(
        nc, in_maps, core_ids=list(range(E)),
        tmpdir=os.environ.get("BASS_TMPDIR"),
    )

    # --- combine: scatter each expert's outputs back to token order ---
    out = np.empty((N_TOK, C), dtype=np.float32)
    for e in range(E):
        yT = LAST_RESULTS.results[e]["yt_out"]  # [C, CAP] f32
        out[idx[e]] = yT[:, :counts[e]].T + b2[e]
        if len(spill[e]):
            z = X[spill[e]].astype(np.float64) @ w1[e].astype(np.float64) + b1[e]
            h = 0.5 * z * (1.0 + _erf(z / np.sqrt(2.0)))
            out[spill[e]] = (h @ w2[e].astype(np.float64) + b2[e]).astype(np.float32)
    return out.reshape(B, T, C)



# revision 2
# speedup vs baseline: 1.0295x; 1.0295x over previous
"""MoE top-1 routing kernel for Trainium2, expert-parallel across 8 NeuronCores.

Strategy (per spec sharding hint): one expert per core. The (tiny) router
runs on host in fp64; tokens are dispatched host-side to their expert's
core (this is the all-to-all dispatch, done during input sharding). Each
core runs a dense FFN  y = gelu(x @ W1 + b1) @ W2  over its tokens in a
fully transposed dataflow:

    hT = W1^T @ xT        (lhsT = W1 slices, rhs = xT slices)
    yT = W2^T @ gelu(hT)  (lhsT = W2 slices, rhs = hT slices)

so the weight matrices are used directly as the stationary operand and no
on-device transposes are needed. Matmuls are bf16 with fp32 PSUM
accumulation; gelu (exact/erf) fused with the b1 bias on the scalar engine.
Outputs are combined host-side (the all-to-all combine) with b2 added on
host.

v2: weights are streamed in m-slab granularity (host pre-packs them into
contiguous [128,512] pieces) so the first real matmul can start ~10us into
the kernel instead of waiting ~40us for the full 18MB preload; w2 streams
on the gpsimd (SWDGE) queue in parallel with the sync-queue w1/x stream;
the PE warmup shrinks from 110 to ~20 matmuls; outputs go out on the sync
(HWDGE) queue for a shorter completion tail.

Shapes are hardcoded for the problem instance:
  x [4,2048,1024] f32, w1 [8,1024,4096], w2 [8,4096,1024], E=8 experts.
"""

import os
import sys

import numpy as np

sys.path.insert(0, "/opt/trn_rl_repo")

import ml_dtypes

try:
    from scipy.special import erf as _erf
except ImportError:          # pragma: no cover
    import math
    _erf = np.vectorize(math.erf)

import concourse.bass as bass
import concourse.mybir as mybir
import concourse.tile as tile
from concourse import bacc
from concourse import bass_utils

B, T, C = 4, 2048, 1024
H, E = 4096, 8
N_TOK = B * T
P = 128                      # partition dim
CAP = 1024                   # per-expert device token capacity; overflow tokens
# (counts above CAP; ~171 for this input) are computed exactly on host
PHASES = [(0, (256, 256)), (512, (256, 256))]
KC = C // P                  # 8  k-tiles over C
KH = H // P                  # 32 k-tiles over H
MH = H // P                  # 32 m-tiles over H (MM1 output partitions)
MC = C // P                  # 8  m-tiles over C (MM2 output partitions)
G1 = 8                       # w1 m-slab groups (4 m-tiles = 512 cols each)
GC = 2                       # w2 mc-slab groups (4 mc-tiles = 512 cols each)
N_WARM = 20                  # PE warm-up matmuls (bridge DMA of x + first slab)

BF16 = mybir.dt.bfloat16
F32 = mybir.dt.float32

_COMPILED = None   # (nc, names) cache so repeat kernel() calls skip rebuild
LAST_RESULTS = None  # bass_utils.BassKernelResults of the last run (for test.py)


def _build_program():
    """Build the single-core Bass/Tile program (SPMD: same program, 8 cores)."""
    nc = bacc.Bacc(
        "TRN2",
        target_bir_lowering=False,
        debug=False,
        enable_asserts=False,
        num_devices=E,
    )

    # DRAM inputs, host-packed so every DMA piece is one contiguous 128KB read:
    #   xt_in rows (ph*KC + k)*P .. +P = xT[k*P:(k+1)*P, ph*512:(ph+1)*512]
    #   w1_in rows (g*KC + k)*P  .. +P = w1[k*P:(k+1)*P, g*512:(g+1)*512]
    #   w2_in rows (gc*KH + kh)*P .. +P = w2[kh*P:(kh+1)*P, gc*512:(gc+1)*512]
    xT_d = nc.dram_tensor("xt_in", [2 * KC * P, 512], BF16, kind="ExternalInput").ap()
    w1_d = nc.dram_tensor("w1_in", [G1 * KC * P, 512], BF16, kind="ExternalInput").ap()
    w2_d = nc.dram_tensor("w2_in", [GC * KH * P, 512], BF16, kind="ExternalInput").ap()
    b1_d = nc.dram_tensor("b1_in", [P, MH], F32, kind="ExternalInput").ap()
    yT_d = nc.dram_tensor("yt_out", [C, CAP], F32, kind="ExternalOutput").ap()

    with tile.TileContext(nc) as tc:
        with (
            tc.tile_pool(name="weights", bufs=1) as wpool,
            tc.tile_pool(name="xt", bufs=1) as xpool,
            tc.tile_pool(name="ht", bufs=1) as hpool,
            tc.tile_pool(name="yout", bufs=4) as ypool,
            tc.tile_pool(name="ps1", bufs=5, space=bass.MemorySpace.PSUM) as ps1pool,
            tc.tile_pool(name="ps2", bufs=3, space=bass.MemorySpace.PSUM) as ps2pool,
        ):
            # --- sync (HWDGE) queue: x phase0, b1, w1 slabs, x phase1 ---
            # Issue order = stream priority; each piece is its own tile so
            # matmul dependencies are per-slab, letting MM1 start after
            # x(p0) + w1 slab g=0 (~2MB) instead of the full 10MB.
            xT_sb = {}           # (ph, k) -> [P, 512] tile
            for k in range(KC):
                t = xpool.tile([P, 512], BF16, tag=f"x0_{k}")
                nc.sync.dma_start(t[:], xT_d[k * P:(k + 1) * P, :])
                xT_sb[0, k] = t
            b1_sb = wpool.tile([P, MH], F32, tag="b1")
            nc.sync.dma_start(b1_sb[:], b1_d[:])
            w1_sb = {}           # (g, k) -> [P, 512] tile
            for g in range(2):
                for k in range(KC):
                    r0 = (g * KC + k) * P
                    t = wpool.tile([P, 512], BF16, tag=f"w1_{g}_{k}")
                    nc.sync.dma_start(t[:], w1_d[r0:r0 + P, :])
                    w1_sb[g, k] = t
            for k in range(KC):
                r0 = (KC + k) * P
                t = xpool.tile([P, 512], BF16, tag=f"x1_{k}")
                nc.sync.dma_start(t[:], xT_d[r0:r0 + P, :])
                xT_sb[1, k] = t
            for g in range(2, G1):
                for k in range(KC):
                    r0 = (g * KC + k) * P
                    t = wpool.tile([P, 512], BF16, tag=f"w1_{g}_{k}")
                    nc.sync.dma_start(t[:], w1_d[r0:r0 + P, :])
                    w1_sb[g, k] = t

            # --- gpsimd (SWDGE) queue: w2 stream, in mc-slab order ---
            w2_sb = {}           # (gc, kh) -> [P, 512] tile
            for gc in range(GC):
                for kh in range(KH):
                    r0 = (gc * KH + kh) * P
                    t = wpool.tile([P, 512], BF16, tag=f"w2_{gc}_{kh}")
                    nc.gpsimd.dma_start(t[:], w2_d[r0:r0 + P, :])
                    w2_sb[gc, kh] = t

            # --- PE warm-up: short matmul burst so the HAM clock-gate is at
            # 2.4 GHz when the first real matmul group becomes runnable
            # (~10us in, once x(p0) + w1 slab 0 have landed) ---
            warm = xpool.tile([P, 256], BF16, tag="warm")
            nc.vector.memset(warm[:], 0.0)
            wps = ps2pool.tile([P, 256], F32, tag="ps2", name="wps")
            for _ in range(N_WARM):
                nc.tensor.matmul(wps[:], warm[:, :P], warm[:], start=True, stop=True)
            # prime the scalar engine's gelu LUT during the DMA window so the
            # first real activation doesn't stall on ACT_TABLE_LOAD
            wact = hpool.tile([P, 8], BF16, tag="wact")
            nc.scalar.activation(wact[:], warm[:, :8],
                                 mybir.ActivationFunctionType.Gelu)

            # --- per phase: MM1+gelu -> hT, then MM2 -> yT. Within a phase,
            # the 2 token blocks are innermost so both matmuls reuse the
            # (m,k) weight tile while it is loaded in the PE array. ---
            for pi, (p0, blocks) in enumerate(PHASES):
                offs = []
                o = 0
                for tn in blocks:
                    offs.append((o, tn))
                    o += tn
                hT = {}
                for m in range(MH):
                    g, j = divmod(m, 4)
                    pss = [ps1pool.tile([P, tn], F32, tag="ps1", name=f"ps1_{pi}_{m}_{i}")
                           for i, (_, tn) in enumerate(offs)]
                    for k in range(KC):
                        for bi, (t0, tn) in enumerate(offs):
                            nc.tensor.matmul(
                                pss[bi][:],
                                w1_sb[g, k][:, j * P:(j + 1) * P],
                                xT_sb[pi, k][:, t0:t0 + tn],
                                start=(k == 0),
                                stop=(k == KC - 1),
                            )
                    for bi, (t0, tn) in enumerate(offs):
                        h = hpool.tile([P, tn], BF16, tag=f"h{m}_{bi}")
                        nc.scalar.activation(
                            h[:], pss[bi][:],
                            mybir.ActivationFunctionType.Gelu,
                            bias=b1_sb[:, m:m + 1],
                        )
                        hT[m, bi] = h
                for mc in range(MC):
                    gc, jc = divmod(mc, 4)
                    pss = [ps2pool.tile([P, tn], F32, tag="ps2", name=f"ps2_{pi}_{mc}_{i}")
                           for i, (_, tn) in enumerate(offs)]
                    for kh in range(KH):
                        for bi in range(len(offs)):
                            nc.tensor.matmul(
                                pss[bi][:],
                                w2_sb[gc, kh][:, jc * P:(jc + 1) * P],
                                hT[kh, bi][:],
                                start=(kh == 0),
                                stop=(kh == KH - 1),
                            )
                    for bi, (t0, tn) in enumerate(offs):
                        y = ypool.tile([P, tn], F32, tag="y")
                        nc.vector.tensor_copy(y[:], pss[bi][:])
                        # outputs ride the sync (HWDGE) queue: it is idle by
                        # now and its completion tail is shorter than SWDGE's
                        nc.sync.dma_start(
                            yT_d[mc * P:(mc + 1) * P,
                                 p0 + t0:p0 + t0 + tn], y[:])

    nc.compile()
    return nc


def _pack_inputs(X, idx_e, count_e, w1_e, w2_e, b1_e):
    """Host-side packing: xT/w1/w2 rearranged so every DMA piece is one
    contiguous [128, 512] slab in DRAM (see _build_program comments)."""
    xT = np.zeros((C, CAP), dtype=ml_dtypes.bfloat16)
    xT[:, :count_e] = X[idx_e].T.astype(ml_dtypes.bfloat16)
    # [C, 1024] -> [ph, k, P, 512] -> rows
    xp = xT.reshape(KC, P, 2, 512).transpose(2, 0, 1, 3).reshape(2 * KC * P, 512)
    # w1 [C, H] -> [g, k, P, 512]: piece (g,k) = w1[k*P:(k+1)*P, g*512:(g+1)*512]
    w1b = w1_e.astype(ml_dtypes.bfloat16)
    w1p = (w1b.reshape(KC, P, G1, 512).transpose(2, 0, 1, 3)
           .reshape(G1 * KC * P, 512))
    # w2 [H, C] -> [gc, kh, P, 512]
    w2b = w2_e.astype(ml_dtypes.bfloat16)
    w2p = (w2b.reshape(KH, P, GC, 512).transpose(2, 0, 1, 3)
           .reshape(GC * KH * P, 512))
    return {
        "xt_in": np.ascontiguousarray(xp),
        "w1_in": np.ascontiguousarray(w1p),
        "w2_in": np.ascontiguousarray(w2p),
        "b1_in": np.ascontiguousarray(b1_e.reshape(MH, P).T),
    }


def kernel(x, w_router, b_router, w1, b1, w2, b2):
    global _COMPILED, LAST_RESULTS

    x = np.asarray(x, dtype=np.float32)
    w_router = np.asarray(w_router, dtype=np.float32)
    b_router = np.asarray(b_router, dtype=np.float32)
    w1 = np.asarray(w1, dtype=np.float32)
    b1 = np.asarray(b1, dtype=np.float32)
    w2 = np.asarray(w2, dtype=np.float32)
    b2 = np.asarray(b2, dtype=np.float32)

    # --- host router (fp64 for a faithful argmax) + top-1 dispatch ---
    X = x.reshape(N_TOK, C)
    logits = X.astype(np.float64) @ w_router.astype(np.float64) + b_router
    top1 = np.argmax(logits, axis=-1)
    idx_all = [np.nonzero(top1 == e)[0] for e in range(E)]
    idx = [i[:CAP] for i in idx_all]          # device share
    spill = [i[CAP:] for i in idx_all]        # host-computed overflow
    counts = [len(i) for i in idx]

    in_maps = [_pack_inputs(X, idx[e], counts[e], w1[e], w2[e], b1[e])
               for e in range(E)]

    if _COMPILED is None:
        _COMPILED = _build_program()
    nc = _COMPILED

    LAST_RESULTS = bass_utils.run_bass_kernel_spmd(
        nc, in_maps, core_ids=list(range(E)),
        tmpdir=os.environ.get("BASS_TMPDIR"),
    )

    # --- combine: scatter each expert's outputs back to token order ---
    out = np.empty((N_TOK, C), dtype=np.float32)
    for e in range(E):
        yT = LAST_RESULTS.results[e]["yt_out"]  # [C, CAP] f32
        out[idx[e]] = yT[:, :counts[e]].T + b2[e]
        if len(spill[e]):
            z = X[spill[e]].astype(np.float64) @ w1[e].astype(np.float64) + b1[e]
            h = 0.5 * z * (1.0 + _erf(z / np.sqrt(2.0)))
            out[spill[e]] = (h @ w2[e].astype(np.float64) + b2[e]).astype(np.float32)
    return out.reshape(B, T, C)


# revision 3
# speedup vs baseline: 1.1019x; 1.0703x over previous
"""MoE top-1 routing kernel for Trainium2, expert-parallel across 8 NeuronCores.

Strategy (per spec sharding hint): one expert per core. The (tiny) router
runs on host in fp64; tokens are dispatched host-side to their expert's
core (this is the all-to-all dispatch, done during input sharding). Each
core runs a dense FFN  y = gelu(x @ W1 + b1) @ W2  over its tokens in a
fully transposed dataflow:

    hT = W1^T @ xT        (lhsT = W1 slices, rhs = xT slices)
    yT = W2^T @ gelu(hT)  (lhsT = W2 slices, rhs = hT slices)

so the weight matrices are used directly as the stationary operand and no
on-device transposes are needed. Matmuls are bf16 with fp32 PSUM
accumulation; gelu (exact/erf) fused with the b1 bias on the scalar engine.
Outputs are combined host-side (the all-to-all combine) with b2 added on
host.

v3: weights/activations are streamed as "k-concatenated" slabs — each SBUF
tile is [128, n*512] holding all k-tiles of one 512-column m-slab side by
side, host-packed so every slab is ONE contiguous DMA with 8-32KB
per-partition descriptors (max DMA efficiency, minimal issue count: 13
input DMAs total). Slab-granular dependencies let the first matmul start
~12us in (after x(phase0) + w1(slab0) = 2MB), with all later slabs
delivered well ahead of PE consumption on a single HWDGE queue. The PE
warmup is sized to bridge exactly that window.

Shapes are hardcoded for the problem instance:
  x [4,2048,1024] f32, w1 [8,1024,4096], w2 [8,4096,1024], E=8 experts.
"""

import os
import sys

import numpy as np

sys.path.insert(0, "/opt/trn_rl_repo")

import ml_dtypes

try:
    from scipy.special import erf as _erf
except ImportError:          # pragma: no cover
    import math
    _erf = np.vectorize(math.erf)

import concourse.bass as bass
import concourse.mybir as mybir
import concourse.tile as tile
from concourse import bacc
from concourse import bass_utils

B, T, C = 4, 2048, 1024
H, E = 4096, 8
N_TOK = B * T
P = 128                      # partition dim
CAP = 1024                   # per-expert device token capacity; overflow tokens
# (counts above CAP; ~171 for this input) are computed exactly on host
PHASES = [(0, (256, 256)), (512, (256, 256))]
KC = C // P                  # 8  k-tiles over C
KH = H // P                  # 32 k-tiles over H
MH = H // P                  # 32 m-tiles over H (MM1 output partitions)
MC = C // P                  # 8  m-tiles over C (MM2 output partitions)
G1 = 8                       # w1 m-slab groups (4 m-tiles = 512 cols each)
GC = 2                       # w2 mc-slab groups (4 mc-tiles = 512 cols each)
N_WARM = 40                  # PE warm-up matmuls (bridge DMA of x + first slab)

BF16 = mybir.dt.bfloat16
F32 = mybir.dt.float32

_COMPILED = None   # (nc, names) cache so repeat kernel() calls skip rebuild
LAST_RESULTS = None  # bass_utils.BassKernelResults of the last run (for test.py)


def _build_program():
    """Build the single-core Bass/Tile program (SPMD: same program, 8 cores)."""
    nc = bacc.Bacc(
        "TRN2",
        target_bir_lowering=False,
        debug=False,
        enable_asserts=False,
        num_devices=E,
    )

    # DRAM inputs, host-packed k-concatenated slabs (one contiguous DMA each):
    #   xt_in  [2*P,  KC*512]: row ph*P+p, col k*512+c = xT[k*P+p, ph*512+c]
    #   w1_in  [G1*P, KC*512]: row g*P+p,  col k*512+c = w1[k*P+p, g*512+c]
    #   w2_in  [GC*P, KH*512]: row gc*P+p, col kh*512+c = w2[kh*P+p, gc*512+c]
    xT_d = nc.dram_tensor("xt_in", [2 * P, KC * 512], BF16, kind="ExternalInput").ap()
    w1_d = nc.dram_tensor("w1_in", [G1 * P, KC * 512], BF16, kind="ExternalInput").ap()
    w2_d = nc.dram_tensor("w2_in", [GC * P, KH * 512], BF16, kind="ExternalInput").ap()
    b1_d = nc.dram_tensor("b1_in", [P, MH], F32, kind="ExternalInput").ap()
    yT_d = nc.dram_tensor("yt_out", [C, CAP], F32, kind="ExternalOutput").ap()

    with tile.TileContext(nc) as tc:
        with (
            tc.tile_pool(name="weights", bufs=1) as wpool,
            tc.tile_pool(name="xt", bufs=1) as xpool,
            tc.tile_pool(name="ht", bufs=1) as hpool,
            tc.tile_pool(name="yout", bufs=4) as ypool,
            tc.tile_pool(name="ps1", bufs=5, space=bass.MemorySpace.PSUM) as ps1pool,
            tc.tile_pool(name="ps2", bufs=3, space=bass.MemorySpace.PSUM) as ps2pool,
        ):
            # --- single sync (HWDGE) queue, in stream-priority order ---
            x_sb = []            # per phase: [P, KC*512]
            t = xpool.tile([P, KC * 512], BF16, tag="x0")
            nc.sync.dma_start(t[:], xT_d[0:P, :])
            x_sb.append(t)
            b1_sb = wpool.tile([P, MH], F32, tag="b1")
            nc.sync.dma_start(b1_sb[:], b1_d[:])
            w1_sb = []           # per g: [P, KC*512]
            for g in range(G1):
                t = wpool.tile([P, KC * 512], BF16, tag=f"w1_{g}")
                nc.sync.dma_start(t[:], w1_d[g * P:(g + 1) * P, :])
                w1_sb.append(t)
            t = xpool.tile([P, KC * 512], BF16, tag="x1")
            nc.sync.dma_start(t[:], xT_d[P:2 * P, :])
            x_sb.append(t)
            w2_sb = []           # per gc: [P, KH*512]
            for gc in range(GC):
                t = wpool.tile([P, KH * 512], BF16, tag=f"w2_{gc}")
                nc.sync.dma_start(t[:], w2_d[gc * P:(gc + 1) * P, :])
                w2_sb.append(t)

            # --- PE warm-up: matmul burst so the HAM clock-gate is at
            # 2.4 GHz when the first real matmul group becomes runnable
            # (~12us in, once x(p0) + w1 slab 0 have landed) ---
            warm = xpool.tile([P, 256], BF16, tag="warm")
            nc.vector.memset(warm[:], 0.0)
            wps = ps2pool.tile([P, 256], F32, tag="ps2", name="wps")
            for _ in range(N_WARM):
                nc.tensor.matmul(wps[:], warm[:, :P], warm[:], start=True, stop=True)
            # prime the scalar engine's gelu LUT during the DMA window so the
            # first real activation doesn't stall on ACT_TABLE_LOAD
            wact = hpool.tile([P, 8], BF16, tag="wact")
            nc.scalar.activation(wact[:], warm[:, :8],
                                 mybir.ActivationFunctionType.Gelu)

            # --- per phase: MM1+gelu -> hT, then MM2 -> yT. Within a phase,
            # the 2 token blocks are innermost so both matmuls reuse the
            # (m,k) weight tile while it is loaded in the PE array. ---
            for pi, (p0, blocks) in enumerate(PHASES):
                offs = []
                o = 0
                for tn in blocks:
                    offs.append((o, tn))
                    o += tn
                hT = {}
                for m in range(MH):
                    g, j = divmod(m, 4)
                    pss = [ps1pool.tile([P, tn], F32, tag="ps1", name=f"ps1_{pi}_{m}_{i}")
                           for i, (_, tn) in enumerate(offs)]
                    for k in range(KC):
                        for bi, (t0, tn) in enumerate(offs):
                            nc.tensor.matmul(
                                pss[bi][:],
                                w1_sb[g][:, k * 512 + j * P:k * 512 + (j + 1) * P],
                                x_sb[pi][:, k * 512 + t0:k * 512 + t0 + tn],
                                start=(k == 0),
                                stop=(k == KC - 1),
                            )
                    for bi, (t0, tn) in enumerate(offs):
                        h = hpool.tile([P, tn], BF16, tag=f"h{m}_{bi}")
                        nc.scalar.activation(
                            h[:], pss[bi][:],
                            mybir.ActivationFunctionType.Gelu,
                            bias=b1_sb[:, m:m + 1],
                        )
                        hT[m, bi] = h
                for mc in range(MC):
                    gc, jc = divmod(mc, 4)
                    pss = [ps2pool.tile([P, tn], F32, tag="ps2", name=f"ps2_{pi}_{mc}_{i}")
                           for i, (_, tn) in enumerate(offs)]
                    for kh in range(KH):
                        for bi in range(len(offs)):
                            nc.tensor.matmul(
                                pss[bi][:],
                                w2_sb[gc][:, kh * 512 + jc * P:kh * 512 + (jc + 1) * P],
                                hT[kh, bi][:],
                                start=(kh == 0),
                                stop=(kh == KH - 1),
                            )
                    y = ypool.tile([P, 512], F32, tag="y")
                    for bi, (t0, tn) in enumerate(offs):
                        nc.vector.tensor_copy(y[:, t0:t0 + tn], pss[bi][:])
                    # outputs ride the sync (HWDGE) queue: it is idle by now
                    # and its completion tail is shorter than SWDGE's
                    nc.sync.dma_start(
                        yT_d[mc * P:(mc + 1) * P, p0:p0 + 512], y[:])

    nc.compile()
    return nc


def _pack_inputs(X, idx_e, count_e, w1_e, w2_e, b1_e):
    """Host-side packing into k-concatenated slabs (see _build_program)."""
    xT = np.zeros((C, CAP), dtype=ml_dtypes.bfloat16)
    xT[:, :count_e] = X[idx_e].T.astype(ml_dtypes.bfloat16)
    # [C, 1024] -> [ph, p, k*512+c]
    xp = (xT.reshape(KC, P, 2, 512).transpose(2, 1, 0, 3)
          .reshape(2 * P, KC * 512))
    # w1 [C, H] -> [g, p, k*512+c]
    w1b = w1_e.astype(ml_dtypes.bfloat16)
    w1p = (w1b.reshape(KC, P, G1, 512).transpose(2, 1, 0, 3)
           .reshape(G1 * P, KC * 512))
    # w2 [H, C] -> [gc, p, kh*512+c]
    w2b = w2_e.astype(ml_dtypes.bfloat16)
    w2p = (w2b.reshape(KH, P, GC, 512).transpose(2, 1, 0, 3)
           .reshape(GC * P, KH * 512))
    return {
        "xt_in": np.ascontiguousarray(xp),
        "w1_in": np.ascontiguousarray(w1p),
        "w2_in": np.ascontiguousarray(w2p),
        "b1_in": np.ascontiguousarray(b1_e.reshape(MH, P).T),
    }


def kernel(x, w_router, b_router, w1, b1, w2, b2):
    global _COMPILED, LAST_RESULTS

    x = np.asarray(x, dtype=np.float32)
    w_router = np.asarray(w_router, dtype=np.float32)
    b_router = np.asarray(b_router, dtype=np.float32)
    w1 = np.asarray(w1, dtype=np.float32)
    b1 = np.asarray(b1, dtype=np.float32)
    w2 = np.asarray(w2, dtype=np.float32)
    b2 = np.asarray(b2, dtype=np.float32)

    # --- host router (fp64 for a faithful argmax) + top-1 dispatch ---
    X = x.reshape(N_TOK, C)
    logits = X.astype(np.float64) @ w_router.astype(np.float64) + b_router
    top1 = np.argmax(logits, axis=-1)
    idx_all = [np.nonzero(top1 == e)[0] for e in range(E)]
    idx = [i[:CAP] for i in idx_all]          # device share
    spill = [i[CAP:] for i in idx_all]        # host-computed overflow
    counts = [len(i) for i in idx]

    in_maps = [_pack_inputs(X, idx[e], counts[e], w1[e], w2[e], b1[e])
               for e in range(E)]

    if _COMPILED is None:
        _COMPILED = _build_program()
    nc = _COMPILED

    LAST_RESULTS = bass_utils.run_bass_kernel_spmd(
        nc, in_maps, core_ids=list(range(E)),
        tmpdir=os.environ.get("BASS_TMPDIR"),
    )

    # --- combine: scatter each expert's outputs back to token order ---
    out = np.empty((N_TOK, C), dtype=np.float32)
    for e in range(E):
        yT = LAST_RESULTS.results[e]["yt_out"]  # [C, CAP] f32
        out[idx[e]] = yT[:, :counts[e]].T + b2[e]
        if len(spill[e]):
            z = X[spill[e]].astype(np.float64) @ w1[e].astype(np.float64) + b1[e]
            h = 0.5 * z * (1.0 + _erf(z / np.sqrt(2.0)))
            out[spill[e]] = (h @ w2[e].astype(np.float64) + b2[e]).astype(np.float32)
    return out.reshape(B, T, C)
